# revision 5
# baseline (speedup 1.0000x reference)
"""Causal attention with bias for B=2,H=16,N=2048,D=128 on 8 trn2 NeuronCores.

Sharding: core c handles heads {2c, 2c+1} for both batches (head-parallel).
The per-head attn_bias (shared across batch) is loaded once per head.

Device kernel computes S^T[j,i] = sum_d k[d,j] q[d,i]*scale per (jblock=128,
ichunk=512) tile, adds the (pre-transposed, causal-masked) bias on DVE,
exponentiates on ACT (no max subtraction needed: logits are O(10)), and runs
PV as attn^T-stationary matmuls against [v | ones] so the softmax denominator
falls out of the ones column. Final normalize is a per-partition
tensor_scalar multiply by the reciprocal row sum.
"""

import numpy as np
import ml_dtypes

import concourse.bass as bass
import concourse.bacc as bacc
import concourse.mybir as mybir
import concourse.tile as tile
from concourse.bass_utils import run_bass_kernel_spmd

B, H, N, D = 2, 16, 2048, 128
NCORES = 8
HPC = H // NCORES          # heads per core
SCALE = float(D) ** -0.5
MASK_VAL = -1e30
CHUNK = 512                # i-chunk width (one psum bank of fp32)
JB = 128                   # j block (partition dim of S^T tiles)
NCHUNK = N // CHUNK        # 4
JPC = CHUNK // JB          # j blocks per chunk on the diagonal: 4
NJB = N // JB              # 16

F32 = mybir.dt.float32
F32R = mybir.dt.float32r
BF16 = mybir.dt.bfloat16

# Which dtype the QK^T matmul runs in. float32r reads the fp32 bits and runs
# at full PE rate for free dim >= 256.
QK_DTYPE = "f32r"          # "f32r" | "f32"
# Fraction of full (non-diagonal) bias-add tiles offloaded from DVE to the PE
# (identity-matmul accumulate into PSUM). Balances DVE vs PE occupancy.
PE_BIAS_EVERY = 0          # every k-th full tile goes to PE; 0 disables


def _qk_cast(ap):
    if QK_DTYPE == "f32r":
        return ap.bitcast(F32R)
    return ap


def build_nc():
    nc = bacc.Bacc(None, target_bir_lowering=False)

    qT_d = nc.dram_tensor("qT", [B, HPC, D, N], F32R, kind="ExternalInput").ap()
    kT_d = nc.dram_tensor("kT", [B, HPC, D, N], F32R, kind="ExternalInput").ap()
    v_d = nc.dram_tensor("v", [B, HPC, N, D], BF16, kind="ExternalInput").ap()
    bias_d = nc.dram_tensor("biasT", [HPC, N, N], F32R, kind="ExternalInput").ap()
    ident_d = nc.dram_tensor("ident", [JB, JB], F32R, kind="ExternalInput").ap()
    out_d = nc.dram_tensor("out", [B, HPC, N, D], F32, kind="ExternalOutput").ap()

    with tile.TileContext(nc) as tc:
        with (
            tc.tile_pool(name="singles", bufs=1) as singles,
            tc.tile_pool(name="kq", bufs=6) as kq_pool,
            tc.tile_pool(name="vp", bufs=3) as v_pool,
            tc.tile_pool(name="bias", bufs=10) as bias_pool,
            tc.tile_pool(name="attn", bufs=40) as attn_pool,
            tc.tile_pool(name="stage", bufs=4) as stage_pool,
            tc.tile_pool(name="small", bufs=8) as small_pool,
            tc.tile_pool(name="ps", bufs=4, space="PSUM") as ps_pool,
            tc.tile_pool(name="po", bufs=3, space="PSUM") as po_pool,
        ):
            ident = singles.tile([JB, JB], F32R, tag="ident", name="ident")
            nc.sync.dma_start(out=ident[:], in_=ident_d)

            pe_bias_ctr = 0

            for hi in range(HPC):
                # per (b, head): whole kT/qT ([d, n]) and v(+ones) resident
                kT_t, qT_t, v_t = {}, {}, {}
                for b in range(B):
                    kT_t[b] = kq_pool.tile([D, N], F32R, tag="kT", name="kT_t")
                    nc.sync.dma_start(out=kT_t[b][:], in_=kT_d[b, hi])
                    qT_t[b] = kq_pool.tile([D, N], F32R, tag="qT", name="qT_t")
                    nc.sync.dma_start(out=qT_t[b][:], in_=qT_d[b, hi])
                    v_t[b] = v_pool.tile([JB, NJB, D + 1], BF16, tag="v", name="v_t")
                    nc.sync.dma_start(
                        out=v_t[b][:, :, 0:D],
                        in_=v_d[b, hi].rearrange("(jb p) d -> p jb d", p=JB),
                    )
                    nc.vector.memset(v_t[b][:, :, D : D + 1], 1.0)

                for c in range(NCHUNK):
                    i0 = c * CHUNK
                    attn_tiles = {}
                    for jb in range(JPC * c + JPC):
                        # columns i < jb*JB within this chunk are fully causal-
                        # masked; skip them.
                        off = max(0, jb * JB - i0)
                        w = CHUNK - off
                        bias_t = bias_pool.tile([JB, CHUNK], F32R, tag="bias", name="bias_t")
                        nc.sync.dma_start(
                            out=bias_t[:, off:],
                            in_=bias_d[hi, jb * JB : (jb + 1) * JB,
                                       i0 + off : i0 + CHUNK],
                        )
                        for b in range(B):
                            ps = ps_pool.tile([JB, CHUNK], F32, tag="ps", name="ps_t")
                            on_pe = (
                                PE_BIAS_EVERY > 0
                                and off == 0
                                and (pe_bias_ctr % PE_BIAS_EVERY == 0)
                            )
                            pe_bias_ctr += 1
                            nc.tensor.matmul(
                                ps[:, off:],
                                lhsT=kT_t[b][:, jb * JB : (jb + 1) * JB],
                                rhs=qT_t[b][:, i0 + off : i0 + CHUNK],
                                start=True,
                                stop=not on_pe,
                            )
                            if on_pe:
                                nc.tensor.matmul(
                                    ps[:, off:],
                                    lhsT=ident[:],
                                    rhs=bias_t[:, off:],
                                    start=False,
                                    stop=True,
                                )
                            else:
                                nc.vector.tensor_add(
                                    ps[:, off:], ps[:, off:], bias_t[:, off:].bitcast(F32)
                                )
                            at = attn_pool.tile([JB, CHUNK], BF16, tag="attn", name="at_t")
                            nc.scalar.activation(
                                at[:, off:],
                                ps[:, off:],
                                mybir.ActivationFunctionType.Exp,
                            )
                            attn_tiles[(b, jb)] = at

                    for b in range(B):
                        stg = stage_pool.tile([JB, JPC, D], F32, tag="stage", name="stg_t")
                        for sub in range(JPC):
                            ib = JPC * c + sub
                            po = po_pool.tile([JB, D + 1], F32, tag="po", name="po_t")
                            for jb in range(ib + 1):
                                nc.tensor.matmul(
                                    po[:],
                                    lhsT=attn_tiles[(b, jb)][
                                        :, sub * JB : (sub + 1) * JB
                                    ],
                                    rhs=v_t[b][:, jb, :],
                                    start=(jb == 0),
                                    stop=(jb == ib),
                                )
                            rc = small_pool.tile([JB, 1], F32, tag="recip", name="rc_t")
                            nc.vector.reciprocal(rc[:], po[:, D : D + 1])
                            nc.vector.tensor_scalar_mul(
                                stg[:, sub, :], po[:, 0:D], rc[:]
                            )
                        nc.sync.dma_start(
                            out=out_d[b, hi, i0 : i0 + CHUNK, :].rearrange(
                                "(s p) d -> p s d", p=JB
                            ),
                            in_=stg[:],
                        )
    nc.finalize()
    return nc


_NC_CACHE = None


def _get_nc():
    global _NC_CACHE
    if _NC_CACHE is None:
        _NC_CACHE = build_nc()
    return _NC_CACHE


def _marshal(q, k, v, attn_bias):
    """Slice/cast/transpose the full inputs into per-core input maps."""
    qs = np.ascontiguousarray(
        np.swapaxes(q.astype(np.float32) * np.float32(SCALE), 2, 3)
    )
    ks = np.ascontiguousarray(np.swapaxes(k.astype(np.float32), 2, 3))
    vb = v.astype(ml_dtypes.bfloat16)
    # biasT[h, j, i] = bias[0, h, i, j] where j <= i else MASK_VAL
    jj = np.arange(N, dtype=np.int32)[:, None]
    ii = np.arange(N, dtype=np.int32)[None, :]
    keep = jj <= ii
    in_maps = []
    for c in range(NCORES):
        h0 = c * HPC
        bt = np.empty((HPC, N, N), dtype=np.float32)
        for hh in range(HPC):
            np.copyto(
                bt[hh],
                np.where(keep, attn_bias[0, h0 + hh].T, np.float32(MASK_VAL)),
            )
        in_maps.append(
            {
                "qT": np.ascontiguousarray(qs[:, h0 : h0 + HPC]),
                "kT": np.ascontiguousarray(ks[:, h0 : h0 + HPC]),
                "v": np.ascontiguousarray(vb[:, h0 : h0 + HPC]),
                "biasT": bt,
                "ident": np.eye(JB, dtype=np.float32),
            }
        )
    return in_maps


def run(q, k, v, attn_bias, trace=False):
    nc = _get_nc()
    in_maps = _marshal(q, k, v, attn_bias)
    res = run_bass_kernel_spmd(
        nc, in_maps, core_ids=list(range(NCORES)), trace=trace
    )
    out = np.empty((B, H, N, D), dtype=np.float32)
    for c in range(NCORES):
        out[:, c * HPC : (c + 1) * HPC] = res.results[c]["out"]
    return out, res


def kernel(q, k, v, mask, attn_bias):
    # mask is all-ones per the input spec; the causal mask is baked into the
    # bias marshaling.
    out, _ = run(
        np.asarray(q), np.asarray(k), np.asarray(v), np.asarray(attn_bias)
    )
    return out


if __name__ == "__main__":
    import reference

    inputs = {kk: np.asarray(vv) for kk, vv in reference.setup_inputs().items()}
    got = kernel(**inputs)
    want = np.asarray(reference.reference(**inputs))
    denom = np.abs(want).max()
    print("abs max err:", np.abs(got - want).max())
    print("rel err:", np.abs(got - want).max() / denom)


# revision 6
# speedup vs baseline: 1.0072x; 1.0072x over previous
"""Causal attention with bias for B=2,H=16,N=2048,D=128 on 8 trn2 NeuronCores.

Sharding: core c handles heads {2c, 2c+1} for both batches (head-parallel).
The per-head attn_bias (shared across batch) is loaded once per head.

Device kernel computes S^T[j,i] = sum_d k[d,j] q[d,i]*scale per (jblock=128,
ichunk=512) tile, adds the (pre-transposed, causal-masked) bias (split
between DVE tensor_add and PE identity-matmul accumulate to balance engines),
exponentiates on ACT (no max subtraction needed: logits are O(10)), and runs
PV as attn^T-stationary matmuls against [v | ones] so the softmax denominator
falls out of the ones column. Final normalize is a per-partition
tensor_scalar multiply by the reciprocal row sum.
"""

import numpy as np
import ml_dtypes

import concourse.bass as bass
import concourse.bacc as bacc
import concourse.mybir as mybir
import concourse.tile as tile
from concourse.bass_utils import run_bass_kernel_spmd

B, H, N, D = 2, 16, 2048, 128
NCORES = 8
HPC = H // NCORES          # heads per core
SCALE = float(D) ** -0.5
MASK_VAL = -30000.0        # exp(x + MASK_VAL) == 0 for any |x| < 1e4
CHUNK = 512                # i-chunk width (one psum bank of fp32)
JB = 128                   # j block (partition dim of S^T tiles)
NCHUNK = N // CHUNK        # 4
JPC = CHUNK // JB          # j blocks per chunk on the diagonal: 4
NJB = N // JB              # 16

F32 = mybir.dt.float32
BF16 = mybir.dt.bfloat16

BIAS_DT = BF16             # attn_bias on-device dtype (BF16 halves DMA)
# every k-th full bias tile is added on the PE (identity matmul accumulate)
# instead of the DVE; balances the two engines. 0 disables.
PE_BIAS_EVERY = 3


def build_nc():
    nc = bacc.Bacc(None, target_bir_lowering=False)

    qT_d = nc.dram_tensor("qT", [B, HPC, D, N], BF16, kind="ExternalInput").ap()
    kT_d = nc.dram_tensor("kT", [B, HPC, D, N], BF16, kind="ExternalInput").ap()
    v_d = nc.dram_tensor("v", [B, HPC, N, D], BF16, kind="ExternalInput").ap()
    bias_d = nc.dram_tensor(
        "biasT", [HPC, N, N], BIAS_DT, kind="ExternalInput"
    ).ap()
    ident_d = nc.dram_tensor("ident", [JB, JB], BF16, kind="ExternalInput").ap()
    out_d = nc.dram_tensor("out", [B, HPC, N, D], F32, kind="ExternalOutput").ap()

    with tile.TileContext(nc) as tc:
        with (
            tc.tile_pool(name="singles", bufs=1) as singles,
            tc.tile_pool(name="kq", bufs=6) as kq_pool,
            tc.tile_pool(name="vp", bufs=3) as v_pool,
            tc.tile_pool(name="bias", bufs=6) as bias_pool,
            tc.tile_pool(name="attn", bufs=16) as attn_pool,
            tc.tile_pool(name="attnd", bufs=12) as attnd_pool,
            tc.tile_pool(name="stage", bufs=4) as stage_pool,
            tc.tile_pool(name="small", bufs=8) as small_pool,
            tc.tile_pool(name="ps", bufs=3, space="PSUM") as ps_pool,
            tc.tile_pool(name="po", bufs=2, space="PSUM") as po_pool,
        ):
            ident = singles.tile([JB, JB], BF16, tag="ident", name="ident")
            nc.sync.dma_start(out=ident[:], in_=ident_d)

            pe_bias_ctr = 0

            for hi in range(HPC):
                # per (b, head): whole kT/qT ([d, n]) and v(+ones) resident
                kT_t, qT_t, v_t = {}, {}, {}
                for b in range(B):
                    kT_t[b] = kq_pool.tile([D, N], BF16, tag="kT", name="kT_t")
                    nc.sync.dma_start(out=kT_t[b][:], in_=kT_d[b, hi])
                    qT_t[b] = kq_pool.tile([D, N], BF16, tag="qT", name="qT_t")
                    nc.sync.dma_start(out=qT_t[b][:], in_=qT_d[b, hi])
                    v_t[b] = v_pool.tile(
                        [JB, NJB, D + 1], BF16, tag="v", name="v_t"
                    )
                    nc.sync.dma_start(
                        out=v_t[b][:, :, 0:D],
                        in_=v_d[b, hi].rearrange("(jb p) d -> p jb d", p=JB),
                    )
                    nc.vector.memset(v_t[b][:, :, D : D + 1], 1.0)

                for c in range(NCHUNK):
                    i0 = c * CHUNK
                    # full j blocks (0..4c-1) processed in pairs sharing a
                    # 2-bank psum tile; diagonal blocks (4c..4c+3) single.
                    attn_full = {}   # (b, pair_idx) -> [JB, 2*CHUNK] blob
                    attn_diag = {}   # (b, k) -> [JB, CHUNK]
                    npairs = (JPC * c) // 2
                    for g in range(npairs):
                        jb0 = 2 * g
                        bias_t = bias_pool.tile(
                            [JB, 2, CHUNK], BIAS_DT, tag="bias", name="bias_t"
                        )
                        nc.sync.dma_start(
                            out=bias_t[:],
                            in_=bias_d[
                                hi, jb0 * JB : (jb0 + 2) * JB, i0 : i0 + CHUNK
                            ].rearrange("(t p) i -> p t i", p=JB),
                        )
                        for b in range(B):
                            ps = ps_pool.tile(
                                [JB, 2 * CHUNK], F32, tag="ps", name="ps_t"
                            )
                            on_pe = (
                                PE_BIAS_EVERY > 0
                                and pe_bias_ctr % PE_BIAS_EVERY == 0
                            )
                            pe_bias_ctr += 1
                            for t in range(2):
                                sl = slice(t * CHUNK, (t + 1) * CHUNK)
                                nc.tensor.matmul(
                                    ps[:, sl],
                                    lhsT=kT_t[b][
                                        :, (jb0 + t) * JB : (jb0 + t + 1) * JB
                                    ],
                                    rhs=qT_t[b][:, i0 : i0 + CHUNK],
                                    start=True,
                                    stop=not on_pe,
                                )
                                if on_pe:
                                    nc.tensor.matmul(
                                        ps[:, sl],
                                        lhsT=ident[:],
                                        rhs=bias_t[:, t, :],
                                        start=False,
                                        stop=True,
                                    )
                            if not on_pe:
                                nc.vector.tensor_add(
                                    ps[:],
                                    ps[:],
                                    bias_t[:].rearrange("p t i -> p (t i)"),
                                )
                            at = attn_pool.tile(
                                [JB, 2 * CHUNK], BF16, tag="attn", name="at_t"
                            )
                            nc.scalar.activation(
                                at[:],
                                ps[:],
                                mybir.ActivationFunctionType.Exp,
                            )
                            attn_full[(b, g)] = at

                    for k in range(JPC):
                        jb = JPC * c + k
                        off = k * JB
                        bias_t = bias_pool.tile(
                            [JB, CHUNK], BIAS_DT, tag="biasd", name="bias_t"
                        )
                        nc.sync.dma_start(
                            out=bias_t[:, off:],
                            in_=bias_d[
                                hi, jb * JB : (jb + 1) * JB,
                                i0 + off : i0 + CHUNK,
                            ],
                        )
                        for b in range(B):
                            ps = ps_pool.tile(
                                [JB, 2 * CHUNK], F32, tag="ps", name="ps_t"
                            )
                            nc.tensor.matmul(
                                ps[:, off:CHUNK],
                                lhsT=kT_t[b][:, jb * JB : (jb + 1) * JB],
                                rhs=qT_t[b][:, i0 + off : i0 + CHUNK],
                                start=True,
                                stop=True,
                            )
                            nc.vector.tensor_add(
                                ps[:, off:CHUNK],
                                ps[:, off:CHUNK],
                                bias_t[:, off:],
                            )
                            at = attnd_pool.tile(
                                [JB, CHUNK], BF16, tag="attnd", name="at_t"
                            )
                            nc.scalar.activation(
                                at[:, off:],
                                ps[:, off:CHUNK],
                                mybir.ActivationFunctionType.Exp,
                            )
                            attn_diag[(b, k)] = at

                    def attn_slice(b, jb, sub):
                        """bf16 attn^T[jb*JB:(jb+1)*JB, i0+sub*JB : ...+JB]"""
                        if jb >= JPC * c:
                            t = attn_diag[(b, jb - JPC * c)]
                            return t[:, sub * JB : (sub + 1) * JB]
                        t = attn_full[(b, jb // 2)]
                        o = (jb % 2) * CHUNK
                        return t[:, o + sub * JB : o + (sub + 1) * JB]

                    for b in range(B):
                        stg = stage_pool.tile(
                            [JB, JPC, D], F32, tag="stage", name="stg_t"
                        )
                        for sub in range(JPC):
                            ib = JPC * c + sub
                            po = po_pool.tile([JB, D + 1], F32, tag="po", name="po_t")
                            for jb in range(ib + 1):
                                nc.tensor.matmul(
                                    po[:],
                                    lhsT=attn_slice(b, jb, sub),
                                    rhs=v_t[b][:, jb, :],
                                    start=(jb == 0),
                                    stop=(jb == ib),
                                )
                            rc = small_pool.tile([JB, 1], F32, tag="recip", name="rc_t")
                            nc.vector.reciprocal(rc[:], po[:, D : D + 1])
                            nc.vector.tensor_scalar_mul(
                                stg[:, sub, :], po[:, 0:D], rc[:]
                            )
                        nc.gpsimd.dma_start(
                            out=out_d[b, hi, i0 : i0 + CHUNK, :].rearrange(
                                "(s p) d -> p s d", p=JB
                            ),
                            in_=stg[:],
                        )
    nc.finalize()
    return nc


_NC_CACHE = None


def _get_nc():
    global _NC_CACHE
    if _NC_CACHE is None:
        _NC_CACHE = build_nc()
    return _NC_CACHE


def _marshal(q, k, v, attn_bias):
    """Slice/cast/transpose the full inputs into per-core input maps."""
    bias_np = ml_dtypes.bfloat16 if BIAS_DT == BF16 else np.float32
    qs = np.ascontiguousarray(
        np.swapaxes(q.astype(np.float32) * np.float32(SCALE), 2, 3)
    ).astype(ml_dtypes.bfloat16)
    ks = np.ascontiguousarray(np.swapaxes(k.astype(np.float32), 2, 3)).astype(
        ml_dtypes.bfloat16
    )
    vb = v.astype(ml_dtypes.bfloat16)
    # biasT[h, j, i] = bias[0, h, i, j] where j <= i else MASK_VAL
    jj = np.arange(N, dtype=np.int32)[:, None]
    ii = np.arange(N, dtype=np.int32)[None, :]
    keep = jj <= ii
    in_maps = []
    for c in range(NCORES):
        h0 = c * HPC
        bt = np.empty((HPC, N, N), dtype=bias_np)
        for hh in range(HPC):
            np.copyto(
                bt[hh],
                np.where(
                    keep, attn_bias[0, h0 + hh].T, np.float32(MASK_VAL)
                ).astype(bias_np),
            )
        in_maps.append(
            {
                "qT": np.ascontiguousarray(qs[:, h0 : h0 + HPC]),
                "kT": np.ascontiguousarray(ks[:, h0 : h0 + HPC]),
                "v": np.ascontiguousarray(vb[:, h0 : h0 + HPC]),
                "biasT": bt,
                "ident": np.eye(JB, dtype=np.float32).astype(ml_dtypes.bfloat16),
            }
        )
    return in_maps


def run(q, k, v, attn_bias, trace=False):
    nc = _get_nc()
    in_maps = _marshal(q, k, v, attn_bias)
    res = run_bass_kernel_spmd(
        nc, in_maps, core_ids=list(range(NCORES)), trace=trace
    )
    out = np.empty((B, H, N, D), dtype=np.float32)
    for c in range(NCORES):
        out[:, c * HPC : (c + 1) * HPC] = res.results[c]["out"]
    return out, res


def kernel(q, k, v, mask, attn_bias):
    # mask is all-ones per the input spec; the causal mask is baked into the
    # bias marshaling.
    out, _ = run(
        np.asarray(q), np.asarray(k), np.asarray(v), np.asarray(attn_bias)
    )
    return out


if __name__ == "__main__":
    import reference

    inputs = {kk: np.asarray(vv) for kk, vv in reference.setup_inputs().items()}
    got = kernel(**inputs)
    want = np.asarray(reference.reference(**inputs))
    denom = np.abs(want).max()
    print("abs max err:", np.abs(got - want).max())
    print("rel err:", np.abs(got - want).max() / denom)


# revision 7
# speedup vs baseline: 1.0508x; 1.0433x over previous
"""Causal attention with bias for B=2,H=16,N=2048,D=128 on 8 trn2 NeuronCores.

Sharding: core c handles heads {2c, 2c+1} for both batches (head-parallel).
The per-head attn_bias (shared across batch) is loaded once per head.

Device kernel computes S^T[j,i] = sum_d k[d,j] q[d,i]*scale per (jblock=128,
ichunk=512) tile, adds the (pre-transposed, causal-masked) bias (split
between DVE tensor_add and PE identity-matmul accumulate to balance engines),
exponentiates on ACT (no max subtraction needed: logits are O(10)), and runs
PV as attn^T-stationary matmuls against [v | ones] so the softmax denominator
falls out of the ones column. Final normalize is a per-partition
tensor_scalar multiply by the reciprocal row sum.
"""

import numpy as np
import ml_dtypes

import concourse.bass as bass
import concourse.bacc as bacc
import concourse.mybir as mybir
import concourse.tile as tile
from concourse.bass_utils import run_bass_kernel_spmd

B, H, N, D = 2, 16, 2048, 128
NCORES = 8
HPC = H // NCORES          # heads per core
SCALE = float(D) ** -0.5
MASK_VAL = -30000.0        # exp(x + MASK_VAL) == 0 for any |x| < 1e4
CHUNK = 512                # i-chunk width (one psum bank of fp32)
JB = 128                   # j block (partition dim of S^T tiles)
NCHUNK = N // CHUNK        # 4
JPC = CHUNK // JB          # j blocks per chunk on the diagonal: 4
NJB = N // JB              # 16

F32 = mybir.dt.float32
BF16 = mybir.dt.bfloat16

BIAS_DT = BF16             # attn_bias on-device dtype (BF16 halves DMA)
# every k-th full bias tile is added on the PE (identity matmul accumulate)
# instead of the DVE; balances the two engines. 0 disables.
PE_BIAS_EVERY = 3


def build_nc():
    nc = bacc.Bacc(None, target_bir_lowering=False)

    qT_d = nc.dram_tensor("qT", [B, HPC, D, N], BF16, kind="ExternalInput").ap()
    kT_d = nc.dram_tensor("kT", [B, HPC, D, N], BF16, kind="ExternalInput").ap()
    v_d = nc.dram_tensor("v", [B, HPC, N, D], BF16, kind="ExternalInput").ap()
    bias_d = nc.dram_tensor(
        "biasT", [HPC, N, N], BIAS_DT, kind="ExternalInput"
    ).ap()
    ident_d = nc.dram_tensor("ident", [JB, JB], BF16, kind="ExternalInput").ap()
    out_d = nc.dram_tensor("out", [B, HPC, N, D], F32, kind="ExternalOutput").ap()

    with tile.TileContext(nc) as tc:
        with (
            tc.tile_pool(name="singles", bufs=1) as singles,
            tc.tile_pool(name="kq", bufs=12) as kq_pool,
            tc.tile_pool(name="vp", bufs=10) as v_pool,
            tc.tile_pool(name="bias", bufs=6) as bias_pool,
            tc.tile_pool(name="attn", bufs=20) as attn_pool,
            tc.tile_pool(name="attnd", bufs=12) as attnd_pool,
            tc.tile_pool(name="stage", bufs=4) as stage_pool,
            tc.tile_pool(name="small", bufs=8) as small_pool,
            tc.tile_pool(name="ps", bufs=3, space="PSUM") as ps_pool,
            tc.tile_pool(name="po", bufs=2, space="PSUM") as po_pool,
        ):
            ident = singles.tile([JB, JB], BF16, tag="ident", name="ident")
            nc.sync.dma_start(out=ident[:], in_=ident_d)

            pe_bias_ctr = 0

            dma_rr = [0]

            def load_dma(out, in_):
                # round-robin DMA issue between the Sync and GpSimd
                # sequencers so neither issue queue serializes the loads
                eng = nc.sync if dma_rr[0] % 2 == 0 else nc.gpsimd
                dma_rr[0] += 1
                eng.dma_start(out=out, in_=in_)

            for hi in range(HPC):
                # kT/qT/v loaded in per-chunk slices, just before first use:
                # kT cols [512c, 512c+512) first used by chunk c (as are qT
                # cols and v row-blocks 4c..4c+3).
                kT_t, qT_t, v_t = {}, {}, {}

                def load_chunk_inputs(cc, hi=hi):
                    for b in range(B):
                        kt = kq_pool.tile([D, CHUNK], BF16, tag="kT", name="kt_t")
                        load_dma(kt[:], kT_d[b, hi, :, cc * CHUNK : (cc + 1) * CHUNK])
                        kT_t[(b, cc)] = kt
                        qt = kq_pool.tile([D, CHUNK], BF16, tag="qT", name="qt_t")
                        load_dma(qt[:], qT_d[b, hi, :, cc * CHUNK : (cc + 1) * CHUNK])
                        qT_t[(b, cc)] = qt
                        vt = v_pool.tile([JB, JPC, D + 1], BF16, tag="v", name="vt_t")
                        load_dma(
                            vt[:, :, 0:D],
                            v_d[b, hi, cc * CHUNK : (cc + 1) * CHUNK, :].rearrange(
                                "(jb p) d -> p jb d", p=JB
                            ),
                        )
                        nc.vector.memset(vt[:, :, D : D + 1], 1.0)
                        v_t[(b, cc)] = vt

                def kT_sl(b, jb):
                    t = kT_t[(b, jb // JPC)]
                    o = (jb % JPC) * JB
                    return t[:, o : o + JB]

                def v_sl(b, jb):
                    return v_t[(b, jb // JPC)][:, jb % JPC, :]

                load_chunk_inputs(0)

                for c in range(NCHUNK):
                    i0 = c * CHUNK
                    # full j blocks (0..4c-1) processed in pairs sharing a
                    # 2-bank psum tile; diagonal blocks (4c..4c+3) single.
                    attn_full = {}   # (b, pair_idx) -> [JB, 2*CHUNK] blob
                    attn_diag = {}   # (b, k) -> [JB, CHUNK]
                    npairs = (JPC * c) // 2
                    for g in range(npairs):
                        jb0 = 2 * g
                        bias_t = bias_pool.tile(
                            [JB, 2, CHUNK], BIAS_DT, tag="bias", name="bias_t"
                        )
                        load_dma(
                            bias_t[:],
                            bias_d[
                                hi, jb0 * JB : (jb0 + 2) * JB, i0 : i0 + CHUNK
                            ].rearrange("(t p) i -> p t i", p=JB),
                        )
                        for b in range(B):
                            ps = ps_pool.tile(
                                [JB, 2 * CHUNK], F32, tag="ps", name="ps_t"
                            )
                            on_pe = (
                                PE_BIAS_EVERY > 0
                                and pe_bias_ctr % PE_BIAS_EVERY == 0
                            )
                            pe_bias_ctr += 1
                            for t in range(2):
                                sl = slice(t * CHUNK, (t + 1) * CHUNK)
                                nc.tensor.matmul(
                                    ps[:, sl],
                                    lhsT=kT_sl(b, jb0 + t),
                                    rhs=qT_t[(b, c)][:],
                                    start=True,
                                    stop=not on_pe,
                                )
                                if on_pe:
                                    nc.tensor.matmul(
                                        ps[:, sl],
                                        lhsT=ident[:],
                                        rhs=bias_t[:, t, :],
                                        start=False,
                                        stop=True,
                                    )
                            if not on_pe:
                                nc.vector.tensor_add(
                                    ps[:],
                                    ps[:],
                                    bias_t[:].rearrange("p t i -> p (t i)"),
                                )
                            at = attn_pool.tile(
                                [JB, 2 * CHUNK], BF16, tag="attn", name="at_t"
                            )
                            nc.scalar.activation(
                                at[:],
                                ps[:],
                                mybir.ActivationFunctionType.Exp,
                            )
                            attn_full[(b, g)] = at

                    for k in range(JPC):
                        jb = JPC * c + k
                        off = k * JB
                        bias_t = bias_pool.tile(
                            [JB, CHUNK], BIAS_DT, tag="biasd", name="bias_t"
                        )
                        load_dma(
                            bias_t[:, off:],
                            bias_d[
                                hi, jb * JB : (jb + 1) * JB,
                                i0 + off : i0 + CHUNK,
                            ],
                        )
                        for b in range(B):
                            ps = ps_pool.tile(
                                [JB, 2 * CHUNK], F32, tag="ps", name="ps_t"
                            )
                            nc.tensor.matmul(
                                ps[:, off:CHUNK],
                                lhsT=kT_sl(b, jb),
                                rhs=qT_t[(b, c)][:, off:],
                                start=True,
                                stop=True,
                            )
                            nc.vector.tensor_add(
                                ps[:, off:CHUNK],
                                ps[:, off:CHUNK],
                                bias_t[:, off:],
                            )
                            at = attnd_pool.tile(
                                [JB, CHUNK], BF16, tag="attnd", name="at_t"
                            )
                            nc.scalar.activation(
                                at[:, off:],
                                ps[:, off:CHUNK],
                                mybir.ActivationFunctionType.Exp,
                            )
                            attn_diag[(b, k)] = at

                    # prefetch the next chunk's (or next head's first) inputs
                    if c + 1 < NCHUNK:
                        load_chunk_inputs(c + 1)
                    def attn_slice(b, jb, sub):
                        """bf16 attn^T[jb*JB:(jb+1)*JB, i0+sub*JB : ...+JB]"""
                        if jb >= JPC * c:
                            t = attn_diag[(b, jb - JPC * c)]
                            return t[:, sub * JB : (sub + 1) * JB]
                        t = attn_full[(b, jb // 2)]
                        o = (jb % 2) * CHUNK
                        return t[:, o + sub * JB : o + (sub + 1) * JB]

                    for b in range(B):
                        stg = stage_pool.tile(
                            [JB, JPC, D], F32, tag="stage", name="stg_t"
                        )
                        for sub in range(JPC):
                            ib = JPC * c + sub
                            po = po_pool.tile([JB, D + 1], F32, tag="po", name="po_t")
                            for jb in range(ib + 1):
                                nc.tensor.matmul(
                                    po[:],
                                    lhsT=attn_slice(b, jb, sub),
                                    rhs=v_sl(b, jb),
                                    start=(jb == 0),
                                    stop=(jb == ib),
                                )
                            rc = small_pool.tile([JB, 1], F32, tag="recip", name="rc_t")
                            nc.vector.reciprocal(rc[:], po[:, D : D + 1])
                            nc.vector.tensor_scalar_mul(
                                stg[:, sub, :], po[:, 0:D], rc[:]
                            )
                        nc.gpsimd.dma_start(
                            out=out_d[b, hi, i0 : i0 + CHUNK, :].rearrange(
                                "(s p) d -> p s d", p=JB
                            ),
                            in_=stg[:],
                        )
    nc.finalize()
    return nc


_NC_CACHE = None


def _get_nc():
    global _NC_CACHE
    if _NC_CACHE is None:
        _NC_CACHE = build_nc()
    return _NC_CACHE


def _marshal(q, k, v, attn_bias):
    """Slice/cast/transpose the full inputs into per-core input maps."""
    bias_np = ml_dtypes.bfloat16 if BIAS_DT == BF16 else np.float32
    qs = np.ascontiguousarray(
        np.swapaxes(q.astype(np.float32) * np.float32(SCALE), 2, 3)
    ).astype(ml_dtypes.bfloat16)
    ks = np.ascontiguousarray(np.swapaxes(k.astype(np.float32), 2, 3)).astype(
        ml_dtypes.bfloat16
    )
    vb = v.astype(ml_dtypes.bfloat16)
    # biasT[h, j, i] = bias[0, h, i, j] where j <= i else MASK_VAL
    jj = np.arange(N, dtype=np.int32)[:, None]
    ii = np.arange(N, dtype=np.int32)[None, :]
    keep = jj <= ii
    in_maps = []
    for c in range(NCORES):
        h0 = c * HPC
        bt = np.empty((HPC, N, N), dtype=bias_np)
        for hh in range(HPC):
            np.copyto(
                bt[hh],
                np.where(
                    keep, attn_bias[0, h0 + hh].T, np.float32(MASK_VAL)
                ).astype(bias_np),
            )
        in_maps.append(
            {
                "qT": np.ascontiguousarray(qs[:, h0 : h0 + HPC]),
                "kT": np.ascontiguousarray(ks[:, h0 : h0 + HPC]),
                "v": np.ascontiguousarray(vb[:, h0 : h0 + HPC]),
                "biasT": bt,
                "ident": np.eye(JB, dtype=np.float32).astype(ml_dtypes.bfloat16),
            }
        )
    return in_maps


def run(q, k, v, attn_bias, trace=False):
    nc = _get_nc()
    in_maps = _marshal(q, k, v, attn_bias)
    res = run_bass_kernel_spmd(
        nc, in_maps, core_ids=list(range(NCORES)), trace=trace
    )
    out = np.empty((B, H, N, D), dtype=np.float32)
    for c in range(NCORES):
        out[:, c * HPC : (c + 1) * HPC] = res.results[c]["out"]
    return out, res


def kernel(q, k, v, mask, attn_bias):
    # mask is all-ones per the input spec; the causal mask is baked into the
    # bias marshaling.
    out, _ = run(
        np.asarray(q), np.asarray(k), np.asarray(v), np.asarray(attn_bias)
    )
    return out


if __name__ == "__main__":
    import reference

    inputs = {kk: np.asarray(vv) for kk, vv in reference.setup_inputs().items()}
    got = kernel(**inputs)
    want = np.asarray(reference.reference(**inputs))
    denom = np.abs(want).max()
    print("abs max err:", np.abs(got - want).max())
    print("rel err:", np.abs(got - want).max() / denom)
